# revision 7
# baseline (speedup 1.0000x reference)
"""GCN v14: dense fp8 count-matrix aggregation, cp-major stream,
split-PSUM overlapped epilogue.

Aggregation agg = S^T h is computed as a dense matmul against a per-core fp8
count matrix S [20224 src, 2500 dst] (counts are small ints - exact in e4m3),
streamed cp-major: 79 chunk-pairs of 256 src rows, each one [128, 5000B]
contiguous-per-partition DMA, consumed by 5 DoubleRow matmuls (N=500 windows)
that share the chunk's LDWEIGHTS. The ifmap's two DoubleRow planes sit at a
2500B stride (2048B-multiple strides halve PE rate; sub-contiguous strided
DMA writes drown the DMA engines in descriptors - both measured).

Epilogue de-tailed by splitting each window's PSUM accumulation: pa_A over
cps [0,56) and pa_B over [56,79). The A-part recip-scales run during the
B-stream (freeing the A banks for the B accumulators: PSUM = 5 accumulators
+ 3 epilogue banks), wself@hT + wneiA@aggA GEMMs for windows 0-2 are
pre-accumulated mid-stream, so after the last chunk only the B-scales +
wneiB GEMMs + relu + store remain (~5us tail vs ~16us in v8). Tail stores
ride the by-then-idle HWDGE queue (SWDGE store dispatch costs ~0.7us each).

Other trims vs v8: dst space 2500/core exact (was 2560 padded), src padded
to 20224 rows = 79 chunk-pairs (was 20480), recip broadcast DMA'd as bf16
(was f32), small constants on the SWDGE queue so the HWDGE queue is a pure
h8+smat pipe. ~55.1MB/core HBM traffic, DMA-roofline bound; fp8 h for the
aggregation path, bf16 h for the self GEMM, fp16 output (rel err ~6.4e-3).
"""

import numpy as np

N_NODES = 20000
D = 128
N_CORES = 8
NPC = 2500                         # dst slots per core (exact, no padding)
N_PAD = NPC * N_CORES
TILE2 = NPC // 5                   # 500-wide psum windows
TPT = 5
NCP = 79                           # src chunk-pairs (256 rows each)
S_PAD = NCP * 256                  # 20224 padded src rows
CPA = 56                           # chunk-pairs in the A-part accumulation

_prog_cache = {}


def _build_program():
    import concourse.mybir as mybir
    from concourse import bacc
    from concourse.tile import TileContext

    dt = mybir.dt
    DR = mybir.MatmulPerfMode.DoubleRow
    nc = bacc.Bacc()

    h8 = nc.declare_dram_parameter("h8", [128, NCP * 256], dt.float8e4, isOutput=False)
    smat = nc.declare_dram_parameter(
        "smat", [128, NCP * 2 * NPC], dt.float8e4, isOutput=False
    )
    hT = nc.declare_dram_parameter("hT", [D, NPC], dt.bfloat16, isOutput=False)
    recip = nc.declare_dram_parameter("recip", [1, NPC], dt.bfloat16, isOutput=False)
    wselfT = nc.declare_dram_parameter("wselfT", [D, D], dt.bfloat16, isOutput=False)
    wneiT = nc.declare_dram_parameter("wneiT", [D, D], dt.bfloat16, isOutput=False)
    bself = nc.declare_dram_parameter("bself", [D, 1], dt.float32, isOutput=False)
    outT = nc.declare_dram_parameter("outT", [D, NPC], dt.float16, isOutput=True)

    NP0 = 20  # cp per h8 piece
    pieces = [(0, 20), (20, 20), (40, 20), (60, 19)]

    with (
        TileContext(nc) as tc,
        tc.tile_pool(name="const", bufs=1) as cpool,
        tc.tile_pool(name="h8p", bufs=4) as hpool,
        tc.tile_pool(name="sel", bufs=6) as spool,
        tc.tile_pool(name="agg", bufs=10) as apool,
        tc.tile_pool(name="res", bufs=3) as opool,
        tc.tile_pool(name="pagg", bufs=5, space="PSUM") as pagg,
        tc.tile_pool(name="pout", bufs=3, space="PSUM") as pout,
    ):
        # h8 pieces lead the HWDGE queue; first agg matmul starts early
        h8p = []
        for lo, n in pieces:
            t = hpool.tile([128, NP0, 2, 128], dt.float8e4)
            nc.sync.dma_start(
                out=t[:, :n, :, :],
                in_=h8[:, lo * 256 : (lo + n) * 256].rearrange(
                    "p (cp two m) -> p cp two m", two=2, m=128
                ),
            )
            h8p.append(t)

        # small constants on SWDGE so they don't delay the smat stream
        hT_sb = cpool.tile([D, NPC], dt.bfloat16)
        nc.gpsimd.dma_start(out=hT_sb[:], in_=hT[:])
        wselfT_sb = cpool.tile([D, D], dt.bfloat16)
        nc.gpsimd.dma_start(out=wselfT_sb[:], in_=wselfT[:])
        wneiT_sb = cpool.tile([D, D], dt.bfloat16)
        nc.gpsimd.dma_start(out=wneiT_sb[:], in_=wneiT[:])
        bself_sb = cpool.tile([D, 1], dt.float32)
        nc.gpsimd.dma_start(out=bself_sb[:], in_=bself[:])
        recip_sb = cpool.tile([128, NPC], dt.bfloat16)
        nc.gpsimd.dma_start(out=recip_sb[:], in_=recip[:, :].to_broadcast([128, NPC]))

        pa_A, pa_B, aggAs, aggBs, pos = {}, {}, {}, {}, {}

        def chunk(cp, lo, hi, pa_map):
            s = spool.tile([128, 2, NPC], dt.float8e4)
            nc.sync.dma_start(
                out=s[:],
                in_=smat[:, cp * 2 * NPC : (cp + 1) * 2 * NPC].rearrange(
                    "p (two n) -> p two n", two=2
                ),
            )
            pc, j = divmod(cp, NP0)
            for k in range(TPT):
                nc.tensor.matmul(
                    out=pa_map[k][:],
                    lhsT=h8p[pc][:, j, :, :],
                    rhs=s[:, :, k * TILE2 : (k + 1) * TILE2],
                    start=(cp == lo),
                    stop=(cp == hi - 1),
                    perf_mode=DR,
                )

        def po_front(k):
            sl = slice(k * TILE2, (k + 1) * TILE2)
            po = pout.tile([128, TILE2], dt.float32, space="PSUM", name="po")
            pos[k] = po
            nc.tensor.matmul(
                out=po[:], lhsT=wselfT_sb[:], rhs=hT_sb[:, sl], start=True, stop=False
            )
            nc.tensor.matmul(
                out=po[:], lhsT=wneiT_sb[:], rhs=aggAs[k][:], start=False, stop=False
            )

        for k in range(TPT):
            pa = pagg.tile([128, TILE2], dt.float32, space="PSUM", name="pa")
            pa_A[k] = pa
        for cp in range(0, CPA):
            chunk(cp, 0, CPA, pa_A)

        # A-part scales free the A banks while the B stream runs
        for k in range(TPT):
            sl = slice(k * TILE2, (k + 1) * TILE2)
            aggA = apool.tile([128, TILE2], dt.bfloat16)
            nc.vector.tensor_mul(out=aggA[:], in0=pa_A[k][:], in1=recip_sb[:, sl])
            aggAs[k] = aggA
            pb = pagg.tile([128, TILE2], dt.float32, space="PSUM", name="pa")
            pa_B[k] = pb

        for cp in range(CPA, NCP):
            chunk(cp, CPA, NCP, pa_B)
            # pre-accumulate wself + wneiA for windows 0-2 mid-stream
            if cp == CPA + 6:
                po_front(0)
            elif cp == CPA + 8:
                po_front(1)
            elif cp == CPA + 10:
                po_front(2)

        for k in range(TPT):
            sl = slice(k * TILE2, (k + 1) * TILE2)
            aggB = apool.tile([128, TILE2], dt.bfloat16)
            nc.vector.tensor_mul(out=aggB[:], in0=pa_B[k][:], in1=recip_sb[:, sl])
            aggBs[k] = aggB
            if k < 3:
                po = pos[k]
            else:
                po = pout.tile([128, TILE2], dt.float32, space="PSUM", name="po")
                nc.tensor.matmul(
                    out=po[:], lhsT=wselfT_sb[:], rhs=hT_sb[:, sl], start=True, stop=False
                )
                nc.tensor.matmul(
                    out=po[:], lhsT=wneiT_sb[:], rhs=aggAs[k][:], start=False, stop=False
                )
            nc.tensor.matmul(
                out=po[:], lhsT=wneiT_sb[:], rhs=aggBs[k][:], start=False, stop=True
            )
            o = opool.tile([128, TILE2], dt.float16)
            nc.scalar.activation(
                out=o[:],
                in_=po[:],
                func=mybir.ActivationFunctionType.Relu,
                bias=bself_sb[:, :1],
            )
            nc.sync.dma_start(out=outT[:, sl], in_=o[:])

    nc.compile()
    return nc


def _host_prep(h, edge_index, deg):
    import ml_dtypes

    f8 = ml_dtypes.float8_e4m3
    bf16 = ml_dtypes.bfloat16

    src = np.asarray(edge_index[0], dtype=np.int64)
    dst = np.asarray(edge_index[1], dtype=np.int64)
    h = np.asarray(h, dtype=np.float32)
    deg = np.asarray(deg, dtype=np.float32)

    h_pad = np.zeros((S_PAD, D), np.float32)
    h_pad[:N_NODES] = h
    h8_flat = (
        h_pad.astype(f8).reshape(NCP, 2, 128, D).transpose(2, 0, 1, 3).reshape(128, -1)
    )
    h8_flat = np.ascontiguousarray(h8_flat)

    recip = np.zeros(N_PAD, np.float32)
    recip[:N_NODES] = 1.0 / np.maximum(deg, 1.0)

    lut = np.arange(256).astype(np.float32).astype(f8)

    core_of_dst = dst // NPC
    order = np.argsort(core_of_dst, kind="stable")
    src_s, dst_s = src[order], dst[order]
    bounds = np.searchsorted(core_of_dst[order], np.arange(N_CORES + 1))

    per_core = []
    for cc in range(N_CORES):
        lo, hi = bounds[cc], bounds[cc + 1]
        s_u8 = np.zeros((S_PAD, NPC), np.uint8)
        np.add.at(s_u8, (src_s[lo:hi], dst_s[lo:hi] - cc * NPC), 1)
        s8 = lut[s_u8]
        s8 = s8.reshape(NCP, 2, 128, NPC).transpose(2, 0, 1, 3).reshape(128, -1)
        per_core.append(np.ascontiguousarray(s8))

    hT_full = np.zeros((N_PAD, D), np.float32)
    hT_full[:N_NODES] = h
    hT_bf = np.ascontiguousarray(hT_full.T.astype(bf16))
    return h8_flat, per_core, recip, hT_bf


def kernel(h, edge_index, deg, w_self, b_self, w_nei):
    import os

    import ml_dtypes
    from concourse.bass_utils import run_bass_kernel_spmd

    bf16 = ml_dtypes.bfloat16

    h8_flat, per_core, recip, hT_bf = _host_prep(h, edge_index, deg)

    wselfT = np.ascontiguousarray(np.asarray(w_self, dtype=np.float32).T.astype(bf16))
    wneiT = np.ascontiguousarray(np.asarray(w_nei, dtype=np.float32).T.astype(bf16))
    b_col = np.ascontiguousarray(np.asarray(b_self, dtype=np.float32).reshape(D, 1))

    in_maps = []
    for cc in range(N_CORES):
        in_maps.append(
            {
                "h8": h8_flat,
                "smat": per_core[cc],
                "hT": np.ascontiguousarray(hT_bf[:, cc * NPC : (cc + 1) * NPC]),
                "recip": np.ascontiguousarray(
                    recip[cc * NPC : (cc + 1) * NPC].reshape(1, NPC).astype(bf16)
                ),
                "wselfT": wselfT,
                "wneiT": wneiT,
                "bself": b_col,
            }
        )

    if "v14" not in _prog_cache:
        _prog_cache["v14"] = _build_program()
    nc = _prog_cache["v14"]

    trace = bool(int(os.environ.get("GCN_TRACE", "0")))
    if "warm" not in _prog_cache:
        # one untraced warmup execution: the first run on a cold device is
        # ~10% slower (HBM/NEFF warmup)
        _prog_cache["warm"] = True
        run_bass_kernel_spmd(nc, in_maps, core_ids=list(range(N_CORES)), trace=False)
    res = run_bass_kernel_spmd(nc, in_maps, core_ids=list(range(N_CORES)), trace=trace)
    kernel.last_results = res

    outT = np.concatenate([r["outT"] for r in res.results], axis=1)
    return np.ascontiguousarray(outT[:, :N_NODES].T.astype(np.float32))


# revision 8
# speedup vs baseline: 1.0053x; 1.0053x over previous
"""GCN v14: dense fp8 count-matrix aggregation, cp-major stream,
split-PSUM overlapped epilogue.

Aggregation agg = S^T h is computed as a dense matmul against a per-core fp8
count matrix S [20224 src, 2500 dst] (counts are small ints - exact in e4m3),
streamed cp-major: 79 chunk-pairs of 256 src rows, each one [128, 5000B]
contiguous-per-partition DMA, consumed by 5 DoubleRow matmuls (N=500 windows)
that share the chunk's LDWEIGHTS. The ifmap's two DoubleRow planes sit at a
2500B stride (2048B-multiple strides halve PE rate; sub-contiguous strided
DMA writes drown the DMA engines in descriptors - both measured).

Epilogue de-tailed by splitting each window's PSUM accumulation: pa_A over
cps [0,56) and pa_B over [56,79). The A-part recip-scales run during the
B-stream (freeing the A banks for the B accumulators: PSUM = 5 accumulators
+ 3 epilogue banks), wself@hT + wneiA@aggA GEMMs for windows 0-2 are
pre-accumulated mid-stream, so after the last chunk only the B-scales +
wneiB GEMMs + relu + store remain (~5us tail vs ~16us in v8). Tail stores
ride the by-then-idle HWDGE queue (SWDGE store dispatch costs ~0.7us each).

Other trims vs v8: dst space 2500/core exact (was 2560 padded), src padded
to 20224 rows = 79 chunk-pairs (was 20480), recip broadcast DMA'd as bf16
(was f32), small constants on the SWDGE queue so the HWDGE queue is a pure
h8+smat pipe. ~55.1MB/core HBM traffic, DMA-roofline bound; fp8 h for the
aggregation path, bf16 h for the self GEMM, fp16 output (rel err ~6.4e-3).
"""

import numpy as np

N_NODES = 20000
D = 128
N_CORES = 8
NPC = 2500                         # dst slots per core (exact, no padding)
N_PAD = NPC * N_CORES
TILE2 = NPC // 5                   # 500-wide psum windows
TPT = 5
NCP = 79                           # src chunk-pairs (256 rows each)
S_PAD = NCP * 256                  # 20224 padded src rows
CPA = 56                           # chunk-pairs in the A-part accumulation

_prog_cache = {}


def _build_program():
    import concourse.mybir as mybir
    from concourse import bacc
    from concourse.tile import TileContext

    dt = mybir.dt
    DR = mybir.MatmulPerfMode.DoubleRow
    nc = bacc.Bacc()

    h8 = nc.declare_dram_parameter("h8", [128, NCP * 256], dt.float8e4, isOutput=False)
    smat = nc.declare_dram_parameter(
        "smat", [128, NCP * 2 * NPC], dt.float8e4, isOutput=False
    )
    hT = nc.declare_dram_parameter("hT", [D, NPC], dt.bfloat16, isOutput=False)
    recip = nc.declare_dram_parameter("recip", [1, NPC], dt.bfloat16, isOutput=False)
    wselfT = nc.declare_dram_parameter("wselfT", [D, D], dt.bfloat16, isOutput=False)
    wneiT = nc.declare_dram_parameter("wneiT", [D, D], dt.bfloat16, isOutput=False)
    bself = nc.declare_dram_parameter("bself", [D, 1], dt.float32, isOutput=False)
    outT = nc.declare_dram_parameter("outT", [D, NPC], dt.float16, isOutput=True)

    NP0 = 20  # cp per h8 piece
    pieces = [(0, 20), (20, 20), (40, 20), (60, 19)]

    with (
        TileContext(nc) as tc,
        tc.tile_pool(name="const", bufs=1) as cpool,
        tc.tile_pool(name="h8p", bufs=4) as hpool,
        tc.tile_pool(name="sel", bufs=6) as spool,
        tc.tile_pool(name="agg", bufs=10) as apool,
        tc.tile_pool(name="res", bufs=3) as opool,
        tc.tile_pool(name="pagg", bufs=5, space="PSUM") as pagg,
        tc.tile_pool(name="pout", bufs=3, space="PSUM") as pout,
    ):
        # h8 piece 0 leads the HWDGE queue; pieces 1-3 are interleaved into
        # the smat stream just before they're needed (cp 20/40/60) so they
        # don't delay the first chunks
        h8p = [hpool.tile([128, NP0, 2, 128], dt.float8e4, name=f"h8p{i}")
               for i in range(len(pieces))]

        def load_piece(i):
            lo, n = pieces[i]
            nc.sync.dma_start(
                out=h8p[i][:, :n, :, :],
                in_=h8[:, lo * 256 : (lo + n) * 256].rearrange(
                    "p (cp two m) -> p cp two m", two=2, m=128
                ),
            )

        load_piece(0)

        # small constants on SWDGE so they don't delay the smat stream
        hT_sb = cpool.tile([D, NPC], dt.bfloat16)
        nc.gpsimd.dma_start(out=hT_sb[:], in_=hT[:])
        wselfT_sb = cpool.tile([D, D], dt.bfloat16)
        nc.gpsimd.dma_start(out=wselfT_sb[:], in_=wselfT[:])
        wneiT_sb = cpool.tile([D, D], dt.bfloat16)
        nc.gpsimd.dma_start(out=wneiT_sb[:], in_=wneiT[:])
        bself_sb = cpool.tile([D, 1], dt.float32)
        nc.gpsimd.dma_start(out=bself_sb[:], in_=bself[:])
        recip_sb = cpool.tile([128, NPC], dt.bfloat16)
        nc.gpsimd.dma_start(out=recip_sb[:], in_=recip[:, :].to_broadcast([128, NPC]))

        pa_A, pa_B, aggAs, aggBs, pos = {}, {}, {}, {}, {}

        def chunk(cp, lo, hi, pa_map):
            s = spool.tile([128, 2, NPC], dt.float8e4)
            nc.sync.dma_start(
                out=s[:],
                in_=smat[:, cp * 2 * NPC : (cp + 1) * 2 * NPC].rearrange(
                    "p (two n) -> p two n", two=2
                ),
            )
            pc, j = divmod(cp, NP0)
            for k in range(TPT):
                nc.tensor.matmul(
                    out=pa_map[k][:],
                    lhsT=h8p[pc][:, j, :, :],
                    rhs=s[:, :, k * TILE2 : (k + 1) * TILE2],
                    start=(cp == lo),
                    stop=(cp == hi - 1),
                    perf_mode=DR,
                )

        def po_front(k):
            sl = slice(k * TILE2, (k + 1) * TILE2)
            po = pout.tile([128, TILE2], dt.float32, space="PSUM", name="po")
            pos[k] = po
            nc.tensor.matmul(
                out=po[:], lhsT=wselfT_sb[:], rhs=hT_sb[:, sl], start=True, stop=False
            )
            nc.tensor.matmul(
                out=po[:], lhsT=wneiT_sb[:], rhs=aggAs[k][:], start=False, stop=False
            )

        for k in range(TPT):
            pa = pagg.tile([128, TILE2], dt.float32, space="PSUM", name="pa")
            pa_A[k] = pa
        for cp in range(0, CPA):
            chunk(cp, 0, CPA, pa_A)
            if cp == 10:
                load_piece(1)
            elif cp == 30:
                load_piece(2)
            elif cp == 50:
                load_piece(3)

        # A-part scales free the A banks while the B stream runs
        for k in range(TPT):
            sl = slice(k * TILE2, (k + 1) * TILE2)
            aggA = apool.tile([128, TILE2], dt.bfloat16)
            nc.vector.tensor_mul(out=aggA[:], in0=pa_A[k][:], in1=recip_sb[:, sl])
            aggAs[k] = aggA
            pb = pagg.tile([128, TILE2], dt.float32, space="PSUM", name="pa")
            pa_B[k] = pb

        for cp in range(CPA, NCP):
            chunk(cp, CPA, NCP, pa_B)
            # pre-accumulate wself + wneiA for windows 0-2 mid-stream
            if cp == CPA + 6:
                po_front(0)
            elif cp == CPA + 8:
                po_front(1)
            elif cp == CPA + 10:
                po_front(2)

        for k in range(TPT):
            sl = slice(k * TILE2, (k + 1) * TILE2)
            aggB = apool.tile([128, TILE2], dt.bfloat16)
            nc.vector.tensor_mul(out=aggB[:], in0=pa_B[k][:], in1=recip_sb[:, sl])
            aggBs[k] = aggB
            if k < 3:
                po = pos[k]
            else:
                po = pout.tile([128, TILE2], dt.float32, space="PSUM", name="po")
                nc.tensor.matmul(
                    out=po[:], lhsT=wselfT_sb[:], rhs=hT_sb[:, sl], start=True, stop=False
                )
                nc.tensor.matmul(
                    out=po[:], lhsT=wneiT_sb[:], rhs=aggAs[k][:], start=False, stop=False
                )
            nc.tensor.matmul(
                out=po[:], lhsT=wneiT_sb[:], rhs=aggBs[k][:], start=False, stop=True
            )
            o = opool.tile([128, TILE2], dt.float16)
            nc.scalar.activation(
                out=o[:],
                in_=po[:],
                func=mybir.ActivationFunctionType.Relu,
                bias=bself_sb[:, :1],
            )
            nc.sync.dma_start(out=outT[:, sl], in_=o[:])

    nc.compile()
    return nc


def _host_prep(h, edge_index, deg):
    import ml_dtypes

    f8 = ml_dtypes.float8_e4m3
    bf16 = ml_dtypes.bfloat16

    src = np.asarray(edge_index[0], dtype=np.int64)
    dst = np.asarray(edge_index[1], dtype=np.int64)
    h = np.asarray(h, dtype=np.float32)
    deg = np.asarray(deg, dtype=np.float32)

    h_pad = np.zeros((S_PAD, D), np.float32)
    h_pad[:N_NODES] = h
    h8_flat = (
        h_pad.astype(f8).reshape(NCP, 2, 128, D).transpose(2, 0, 1, 3).reshape(128, -1)
    )
    h8_flat = np.ascontiguousarray(h8_flat)

    recip = np.zeros(N_PAD, np.float32)
    recip[:N_NODES] = 1.0 / np.maximum(deg, 1.0)

    lut = np.arange(256).astype(np.float32).astype(f8)

    core_of_dst = dst // NPC
    order = np.argsort(core_of_dst, kind="stable")
    src_s, dst_s = src[order], dst[order]
    bounds = np.searchsorted(core_of_dst[order], np.arange(N_CORES + 1))

    per_core = []
    for cc in range(N_CORES):
        lo, hi = bounds[cc], bounds[cc + 1]
        s_u8 = np.zeros((S_PAD, NPC), np.uint8)
        np.add.at(s_u8, (src_s[lo:hi], dst_s[lo:hi] - cc * NPC), 1)
        s8 = lut[s_u8]
        s8 = s8.reshape(NCP, 2, 128, NPC).transpose(2, 0, 1, 3).reshape(128, -1)
        per_core.append(np.ascontiguousarray(s8))

    hT_full = np.zeros((N_PAD, D), np.float32)
    hT_full[:N_NODES] = h
    hT_bf = np.ascontiguousarray(hT_full.T.astype(bf16))
    return h8_flat, per_core, recip, hT_bf


def kernel(h, edge_index, deg, w_self, b_self, w_nei):
    import os

    import ml_dtypes
    from concourse.bass_utils import run_bass_kernel_spmd

    bf16 = ml_dtypes.bfloat16

    h8_flat, per_core, recip, hT_bf = _host_prep(h, edge_index, deg)

    wselfT = np.ascontiguousarray(np.asarray(w_self, dtype=np.float32).T.astype(bf16))
    wneiT = np.ascontiguousarray(np.asarray(w_nei, dtype=np.float32).T.astype(bf16))
    b_col = np.ascontiguousarray(np.asarray(b_self, dtype=np.float32).reshape(D, 1))

    in_maps = []
    for cc in range(N_CORES):
        in_maps.append(
            {
                "h8": h8_flat,
                "smat": per_core[cc],
                "hT": np.ascontiguousarray(hT_bf[:, cc * NPC : (cc + 1) * NPC]),
                "recip": np.ascontiguousarray(
                    recip[cc * NPC : (cc + 1) * NPC].reshape(1, NPC).astype(bf16)
                ),
                "wselfT": wselfT,
                "wneiT": wneiT,
                "bself": b_col,
            }
        )

    if "v14" not in _prog_cache:
        _prog_cache["v14"] = _build_program()
    nc = _prog_cache["v14"]

    trace = bool(int(os.environ.get("GCN_TRACE", "0")))
    if "warm" not in _prog_cache:
        # one untraced warmup execution: the first run on a cold device is
        # ~10% slower (HBM/NEFF warmup)
        _prog_cache["warm"] = True
        run_bass_kernel_spmd(nc, in_maps, core_ids=list(range(N_CORES)), trace=False)
    res = run_bass_kernel_spmd(nc, in_maps, core_ids=list(range(N_CORES)), trace=trace)
    kernel.last_results = res

    outT = np.concatenate([r["outT"] for r in res.results], axis=1)
    return np.ascontiguousarray(outT[:, :N_NODES].T.astype(np.float32))
